# revision 62
# baseline (speedup 1.0000x reference)
"""Int4-quantized column-parallel linear (LLaMA-7B FFN up-proj) on 8 TRN2 cores.

y[b,s,o] = sum_i x[b,s,i] * (unpack_int4(weight_q)[o,i] * scale[o]) + bias[o]

Strategy (per core, 1/8 of out_features = 1376):
  - int4 weight nibbles are exact in fp8 e4m3; x is split x ~= hi + lo with
    hi = fp8(x), lo = fp8(x - hi), so the pair of fp8 matmuls reconstructs x
    to ~8e-4 relative. The lo-correction is skipped for LO_SKIP of the 16
    k-steps (residual error 2.65%*sqrt(LO_SKIP/16), far inside the 2e-2 gate),
    saving 1/32 of PE cycles per skipped step.
  - all matmuls run in MatmulPerfMode.DoubleRow (fp8 double-pumping): one
    instruction contracts 2x128 K rows at 0.5 PE cycles per output row.
  - layout: hi8/lo8 are converted straight from f32 x (no fp16 step), then
    DMA-transposed as uint16 byte-pairs; the resulting pair-interleaved
    [k-pair, 2, tok] operand rides the MOVING side of the matmul (the ISA
    allows strided moving APs but not strided LDWEIGHTS), with the weights
    pre-deinterleaved once into the matching pair-contiguous stationary
    layout. Output lands PSUM-[feat, tok], is drained with a fused per-
    partition scale*psum+bias tensor_scalar to fp16, DMA-transposed back to
    [tok, feat], and converted to f32.
"""

from contextlib import ExitStack

import numpy as np

import concourse.bass as bass
import concourse.tile as tile
from concourse import bacc, mybir

F32 = mybir.dt.float32
F16 = mybir.dt.float16
F8 = mybir.dt.float8e4
I32 = mybir.dt.int32
U16 = mybir.dt.uint16

B, S, IN, OUT = 4, 2048, 4096, 11008
NCORES = 8
TOK = B * S
FEAT = OUT // NCORES

P = 128
KP = IN // P          # 32 k-subtiles of 128
KSTEP = KP // 2       # 16 DoubleRow k-steps of 256
NTOK = TOK // P       # 64 token tiles
HALF = IN // 2

# Number of 256-row k-steps whose lo-correction matmul is skipped (of KSTEP).
LO_SKIP = 3

# Pool depths for the x pipeline (referenced by the manual WAR dep edges).
HI_BUFS = 2
LO_BUFS = 3
XT_BUFS = 5


def _dep(from_inst, to_inst, reason):
    tile.add_dep_helper(from_inst.ins, to_inst.ins, reason=reason)


def build(tok=TOK, in_dim=IN, feat=FEAT, debug_xt=False):
    assert tok % P == 0 and in_dim % 256 == 0
    ntok = tok // P
    kp = in_dim // P
    kstep = kp // 2
    half = in_dim // 2
    fchunks = [(c, c * P, min(P, feat - c * P)) for c in range((feat + P - 1) // P)]
    nfc = len(fchunks)

    nc = bacc.Bacc("TRN2", target_bir_lowering=False, debug=False,
                   num_devices=NCORES)
    x_d = nc.dram_tensor("x", [tok, in_dim], F32, kind="ExternalInput").ap()
    wq_d = nc.dram_tensor("wq", [feat, half], I32, kind="ExternalInput").ap()
    sc_d = nc.dram_tensor("scale", [feat], F32, kind="ExternalInput").ap()
    bi_d = nc.dram_tensor("bias", [feat], F32, kind="ExternalInput").ap()
    y_d = nc.dram_tensor("y", [tok, feat], F32, kind="ExternalOutput").ap()
    if debug_xt:
        dxh_d = nc.dram_tensor("dxh", [ntok, P, kp // 2, P], U16,
                               kind="ExternalOutput").ap()
        dxl_d = nc.dram_tensor("dxl", [ntok, P, kp // 2, P], U16,
                               kind="ExternalOutput").ap()

    with tile.TileContext(nc) as tc, ExitStack() as ctx:
        const = ctx.enter_context(tc.tile_pool(name="const", bufs=1))
        wtp = ctx.enter_context(tc.tile_pool(name="wt", bufs=1))
        wstg = ctx.enter_context(tc.tile_pool(name="wstg", bufs=2))
        wqp = ctx.enter_context(tc.tile_pool(name="wqp", bufs=1))
        wbp = ctx.enter_context(tc.tile_pool(name="wbp", bufs=2))
        wtmp = ctx.enter_context(tc.tile_pool(name="wtmp", bufs=2))
        x32p = ctx.enter_context(tc.tile_pool(name="x32", bufs=2))
        hip = ctx.enter_context(tc.tile_pool(name="hi8", bufs=HI_BUFS))
        lop = ctx.enter_context(tc.tile_pool(name="lo8", bufs=LO_BUFS))
        xthp = ctx.enter_context(tc.tile_pool(name="xth", bufs=XT_BUFS))
        xtlp = ctx.enter_context(tc.tile_pool(name="xtl", bufs=XT_BUFS))
        otp = ctx.enter_context(tc.tile_pool(name="ot16", bufs=2))
        ytp = ctx.enter_context(tc.tile_pool(name="yt16", bufs=2))
        y32p = ctx.enter_context(tc.tile_pool(name="y32", bufs=2))
        pout = ctx.enter_context(tc.tile_pool(name="pout", bufs=2, space="PSUM"))

        # scale/bias arranged (partition, chunk): [p, c] = value[128c + p];
        # pad partitions of the last chunk are zeroed so every chunk drains a
        # full 128 partitions from initialized memory
        scale_sb = const.tile([P, nfc], F32)
        bias_sb = const.tile([P, nfc], F32)
        lastsz = feat - (nfc - 1) * P
        for src, dst in ((sc_d, scale_sb), (bi_d, bias_sb)):
            if lastsz < P:
                nc.vector.memset(dst[lastsz:, nfc - 1:nfc], 0.0)
            nc.sync.dma_start(
                out=dst[:, :nfc - 1],
                in_=src[:(nfc - 1) * P].rearrange("(c p) -> p c", p=P))
            nc.sync.dma_start(
                out=dst[:lastsz, nfc - 1:nfc],
                in_=src[(nfc - 1) * P:].rearrange("(c p) -> p c", c=1))

        # Stationary weights, pair-contiguous: [k-pair(part), kblk, i, feat]
        # where element (p, j, i, f) = w[f, 256j + 2p + i]. The feat axis is
        # padded to nfc*128 with zero weights so every matmul/drain covers a
        # full 128 output partitions (cost is per output ROW, so this is free)
        feat_pad = nfc * P
        wT8x = wtp.tile([P, kstep, 2, feat_pad], F8)
        if feat_pad > feat:
            nc.gpsimd.memset(wT8x[:, :, :, feat:], 0.0)

        # ---- Phase W: unpack -> fp8 -> u16-pair transpose -> deinterleave ----
        # Nibbles sign-extended in 2 fused ALU ops ((q<<28)>>a28 / (q<<24)>>a28)
        # with the int32 ALU result converted to fp8 by the strided write.
        wq_tiles = {}

        def emit_wload(ftidx, f0, fsz):
            wqt = wqp.tile([P, half], I32)
            ldq = (nc.sync, nc.scalar, nc.gpsimd)[ftidx % 3]
            ldq.dma_start(out=wqt[:fsz], in_=wq_d[f0:f0 + fsz])
            wq_tiles[ftidx] = wqt

        wphase = {}

        def emit_wtile(ftidx, f0, fsz):
            wqt = wq_tiles.pop(ftidx)
            wb8 = wbp.tile([P, in_dim], F8)
            wb8v = wb8[:fsz].rearrange("p (i two) -> p two i", two=2)
            # bitVec TSP ops cannot cast and only run on DVE; bias the nibbles
            # in int32 there, then convert+subtract-8 on ACT (Copy, float bias)
            n_lo = wtmp.tile([P, half], I32)
            nc.vector.tensor_scalar(
                out=n_lo[:fsz], in0=wqt[:fsz], scalar1=15, scalar2=8,
                op0=mybir.AluOpType.bitwise_and, op1=mybir.AluOpType.bitwise_xor)
            n_hi = wtmp.tile([P, half], I32)
            nc.vector.tensor_scalar(
                out=n_hi[:fsz], in0=wqt[:fsz], scalar1=4, scalar2=8,
                op0=mybir.AluOpType.logical_shift_right,
                op1=mybir.AluOpType.bitwise_xor)
            c0 = nc.scalar.activation(out=wb8v[:, 0], in_=n_lo[:fsz],
                                      func=mybir.ActivationFunctionType.Copy,
                                      bias=-8.0)
            c1 = nc.scalar.activation(out=wb8v[:, 1], in_=n_hi[:fsz],
                                      func=mybir.ActivationFunctionType.Copy,
                                      bias=-8.0)
            if ftidx >= 2:  # wbp bufs=2: WAR vs the bitcast transpose read
                _dep(c0, wphase[ftidx - 2]["wtr"],
                                    reason="wb8 buffer WAR vs bitcast read")
                _dep(c1, wphase[ftidx - 2]["wtr"],
                                    reason="wb8 buffer WAR vs bitcast read")
            stg = wstg.tile([P, kstep, P], U16)
            # all DmaTransposeAnt share the ACT queue: concurrent transposes
            # on different hwdge queues corrupt each other (shared xbar)
            wtr = nc.scalar.dma_start_transpose(out=stg[:, :, :fsz],
                                                in_=wb8[:fsz].bitcast(U16))
            _dep(wtr, c0, reason="w transpose reads wb8 bitcast")
            _dep(wtr, c1, reason="w transpose reads wb8 bitcast")
            if ftidx >= 2:  # wstg bufs=2: WAR vs the bitcast deint reads
                for d in wphase[ftidx - 2]["deints"]:
                    _dep(wtr, d,
                                        reason="stg buffer WAR vs bitcast read")
            # deinterleave the (k, k+1) byte pairs into the stationary layout
            sv = stg[:].rearrange("p a b -> p (a b)").bitcast(F8).rearrange(
                "p (j f two) -> p j two f", j=kstep, two=2)
            deints = []
            for b in range(2):
                d = nc.gpsimd.tensor_copy(out=wT8x[:, :, b, f0:f0 + fsz],
                                          in_=sv[:, :, b, :fsz])
                _dep(d, wtr, reason="deint reads stg bitcast")
                deints.append(d)
            wphase[ftidx] = {"wtr": wtr, "deints": deints}

        # ---- Main loop stages ----
        # The dependency tracker does not see accesses made through bitcast
        # views, so every such read/write gets an explicit add_dep_helper edge
        # (RAW: view-reader after producer; WAR: buffer re-writer after the
        # last view-reader).
        state = {}

        def emit_load(i):
            x32 = x32p.tile([P, in_dim], F32)
            nc.sync.dma_start(out=x32[:], in_=x_d[i * P:(i + 1) * P])
            state[i] = {"x32": x32}

        def emit_convert(i):
            st = state[i]
            x32 = st["x32"]
            hi8 = hip.tile([P, in_dim], F8)
            cvt = nc.scalar.activation(out=hi8[:], in_=x32[:],
                                       func=mybir.ActivationFunctionType.Copy)
            on_dve = False
            if i >= HI_BUFS:
                _dep(cvt, state[i - HI_BUFS]["thi"],
                     reason="hi8 buffer WAR vs bitcast T_hi read")
            st["hi8"], st["cvt"] = hi8, cvt
            if not on_dve:
                # T_hi issued on ACT right after the ACT cvt (same engine, no
                # sequencer stall); RAW through the bitcast view
                emit_thi(i)
            lo8 = lop.tile([P, in_dim], F8)
            sub = nc.vector.tensor_tensor(out=lo8[:], in0=x32[:], in1=hi8[:],
                                          op=mybir.AluOpType.subtract)
            if i >= LO_BUFS:
                _dep(sub, state[i - LO_BUFS]["tlo"],
                     reason="lo8 buffer WAR vs bitcast T_lo read")
            st["lo8"], st["sub"] = lo8, sub

        def emit_thi(i):
            st = state[i]
            xth = xthp.tile([P, kstep, P], U16)
            thi = nc.scalar.dma_start_transpose(out=xth[:],
                                                in_=st["hi8"][:].bitcast(U16))
            _dep(thi, st["cvt"], reason="T_hi reads hi8 via bitcast")
            if i >= XT_BUFS:
                _dep(thi, state[i - XT_BUFS]["mm_last"],
                     reason="xth buffer WAR vs bitcast readers")
            if debug_xt:
                nc.gpsimd.dma_start(out=dxh_d[i], in_=xth[:])
            st["xth"] = xth
            st["thi"] = thi

        def emit_tlo(i):
            # lo transpose one iteration later: lo8(i) is long done, so the
            # ACT sequencer never blocks on the DVE semaphore
            st = state[i]
            if "xth" not in st:
                emit_thi(i)  # DVE-converted tiles get their T_hi here too
            xtl = xtlp.tile([P, kstep, P], U16)
            tlo = nc.scalar.dma_start_transpose(out=xtl[:],
                                                in_=st["lo8"][:].bitcast(U16))
            _dep(tlo, st["sub"], reason="T_lo reads lo8 via bitcast")
            if i >= XT_BUFS:
                _dep(tlo, state[i - XT_BUFS]["mm_last"],
                     reason="xtl buffer WAR vs bitcast readers")
            if debug_xt:
                nc.gpsimd.dma_start(out=dxl_d[i], in_=xtl[:])
            st["xtl"] = xtl
            st["tlo"] = tlo

        def emit_mm(i, po):
            st = state[i]
            st["po"] = po
            vh = st["xth"][:].rearrange("p a b -> p (a b)").bitcast(F8).rearrange(
                "p (j t two) -> p j two t", j=kstep, two=2)
            vl = st["xtl"][:].rearrange("p a b -> p (a b)").bitcast(F8).rearrange(
                "p (j t two) -> p j two t", j=kstep, two=2)
            first = None
            last = None
            for c, f0, fsz in fchunks:
                for j in range(kstep):
                    for b, v in ((0, vh), (1, vl)):
                        if b == 1 and j < LO_SKIP:
                            continue
                        last = nc.tensor.matmul(
                            out=po[:, c, :],
                            lhsT=wT8x[:, j, :, f0:f0 + P],
                            rhs=v[:, j, :, :],
                            start=(j == 0 and b == 0),
                            stop=(j == kstep - 1 and b == 1),
                            perf_mode=mybir.MatmulPerfMode.DoubleRow)
                        if first is None:
                            first = last
            # RAW: matmuls read xth/xtl via bitcast views (PE is in-order, so
            # an edge on the first matmul covers the whole tile)
            _dep(first, st["thi"], reason="mm reads xth bitcast")
            _dep(first, st["tlo"], reason="mm reads xtl bitcast")
            st["mm_last"] = last

        def emit_drainblock(i):
            # one iteration after the matmuls: all chunk stops are long past,
            # so these head-of-stream DVE ops run immediately and release the
            # PSUM buffer well before its next writer needs it
            st = state[i]
            po = st["po"]
            ot16 = otp.tile([P, nfc, P], F16)
            st["ot16"] = ot16
            for c, f0, fsz in fchunks:
                nc.vector.tensor_scalar(
                    out=ot16[:, c, :], in0=po[:, c, :],
                    scalar1=scale_sb[:, c:c + 1],
                    scalar2=bias_sb[:, c:c + 1],
                    op0=mybir.AluOpType.mult, op1=mybir.AluOpType.add)

        def emit_ytail(i):
            # one iteration after the drain: ot16(i) is complete, so the SP
            # sequencer never blocks waiting on DVE before issuing the x load
            ot16 = state[i]["ot16"]
            yt16 = ytp.tile([P, nfc, P], F16)
            nc.scalar.dma_start_transpose(
                out=yt16[:], in_=ot16[:].rearrange("p a b -> p (a b)"))
            y32 = y32p.tile([P, feat], F32)
            nc.gpsimd.tensor_copy(
                out=y32[:], in_=yt16[:].rearrange("p a b -> p (a b)")[:, :feat])
            nc.gpsimd.dma_start(out=y_d[i * P:(i + 1) * P, :], in_=y32[:])

        wdims = [(c * P, min(P, feat - c * P)) for c in range(nfc)]
        for ftidx in range(nfc + 2):
            if ftidx < nfc:
                emit_wload(ftidx, *wdims[ftidx])
            if ftidx >= 2:
                emit_wtile(ftidx - 2, *wdims[ftidx - 2])

        for i in range(ntok + 6):
            if 4 <= i <= ntok + 3:
                emit_drainblock(i - 4)
            if 5 <= i <= ntok + 4:
                emit_ytail(i - 5)
            if i < ntok:
                emit_load(i)
            if 1 <= i <= ntok:
                emit_convert(i - 1)
            if 2 <= i <= ntok + 1:
                emit_tlo(i - 2)
            if 3 <= i <= ntok + 2:
                po = pout.tile([P, nfc, P], F32)
                emit_mm(i - 3, po)
            if i >= 6:
                del state[i - 6]

    nc.compile()
    return nc


_CACHE = {}


def _get_program():
    if "nc" not in _CACHE:
        _CACHE["nc"] = build()
    return _CACHE["nc"]


def kernel(x, weight_q, scale, bias):
    from concourse.bass_utils import run_bass_kernel_spmd

    try:
        import jax

        jax.config.update("jax_compilation_cache_dir", "/root/problem/jax_cache")
        jax.config.update("jax_persistent_cache_min_compile_time_secs", 0)
    except Exception:
        pass

    nc = _get_program()
    xr = np.ascontiguousarray(np.asarray(x, dtype=np.float32).reshape(TOK, IN))
    wq = np.asarray(weight_q, dtype=np.int32)
    sc = np.asarray(scale, dtype=np.float32)
    bi = np.asarray(bias, dtype=np.float32)
    in_maps = []
    for c in range(NCORES):
        f0 = c * FEAT
        in_maps.append({
            "x": xr,
            "wq": np.ascontiguousarray(wq[f0:f0 + FEAT]),
            "scale": np.ascontiguousarray(sc[f0:f0 + FEAT]),
            "bias": np.ascontiguousarray(bi[f0:f0 + FEAT]),
        })
    res = run_bass_kernel_spmd(nc, in_maps, list(range(NCORES))).results
    y = np.concatenate([res[c]["y"] for c in range(NCORES)], axis=1)
    return y.reshape(B, S, OUT)


# revision 73
# speedup vs baseline: 1.4254x; 1.4254x over previous
"""Int4-quantized column-parallel linear (LLaMA-7B FFN up-proj) on 8 TRN2 cores.

y[b,s,o] = sum_i x[b,s,i] * (unpack_int4(weight_q)[o,i] * scale[o]) + bias[o]

Strategy (per core, 1/8 of out_features = 1376):
  - int4 weight nibbles are exact in fp8 e4m3; x is split x ~= hi + lo with
    hi = fp8(x), lo = fp8(x - hi), so the pair of fp8 matmuls reconstructs x
    to ~8e-4 relative. The lo-correction is skipped for LO_SKIP of the 16
    k-steps (residual error 2.65%*sqrt(LO_SKIP/16), far inside the 2e-2 gate),
    saving 1/32 of PE cycles per skipped step.
  - all matmuls run in MatmulPerfMode.DoubleRow (fp8 double-pumping): one
    instruction contracts 2x128 K rows at 0.5 PE cycles per output row.
  - layout: hi8/lo8 are converted straight from f32 x (no fp16 step), then
    DMA-transposed as uint16 byte-pairs; the resulting pair-interleaved
    [k-pair, 2, tok] operand rides the MOVING side of the matmul (the ISA
    allows strided moving APs but not strided LDWEIGHTS), with the weights
    pre-deinterleaved once into the matching pair-contiguous stationary
    layout. Output lands PSUM-[feat, tok], is drained with a fused per-
    partition scale*psum+bias tensor_scalar to fp16, DMA-transposed back to
    [tok, feat], and converted to f32.
"""

from contextlib import ExitStack

import numpy as np

import concourse.bass as bass
import concourse.tile as tile
from concourse import bacc, mybir

F32 = mybir.dt.float32
F16 = mybir.dt.float16
F8 = mybir.dt.float8e4
I32 = mybir.dt.int32
U16 = mybir.dt.uint16

B, S, IN, OUT = 4, 2048, 4096, 11008
NCORES = 8
TOK = B * S
FEAT = OUT // NCORES

P = 128
KP = IN // P          # 32 k-subtiles of 128
KSTEP = KP // 2       # 16 DoubleRow k-steps of 256
NTOK = TOK // P       # 64 token tiles
HALF = IN // 2

# Number of 256-row k-steps whose lo-correction matmul is skipped (of KSTEP).
LO_SKIP = 3

# Pool depths for the x pipeline (referenced by the manual WAR dep edges).
HI_BUFS = 2
LO_BUFS = 3
XT_BUFS = 5
XTL_BUFS = 3
WB_BUFS = 2
WSTG_BUFS = 1


def _dep(from_inst, to_inst, reason):
    tile.add_dep_helper(from_inst.ins, to_inst.ins, reason=reason)


def build(tok=TOK, in_dim=IN, feat=FEAT, debug_xt=False):
    assert tok % P == 0 and in_dim % 256 == 0
    ntok = tok // P
    kp = in_dim // P
    kstep = kp // 2
    half = in_dim // 2
    fchunks = [(c, c * P, min(P, feat - c * P)) for c in range((feat + P - 1) // P)]
    nfc = len(fchunks)

    nc = bacc.Bacc("TRN2", target_bir_lowering=False, debug=False,
                   num_devices=NCORES)
    x_d = nc.dram_tensor("x", [tok, in_dim], F32, kind="ExternalInput").ap()
    wq_d = nc.dram_tensor("wq", [feat, half], I32, kind="ExternalInput").ap()
    sc_d = nc.dram_tensor("scale", [feat], F32, kind="ExternalInput").ap()
    bi_d = nc.dram_tensor("bias", [feat], F32, kind="ExternalInput").ap()
    y_d = nc.dram_tensor("y", [tok, feat], F32, kind="ExternalOutput").ap()
    if debug_xt:
        dxh_d = nc.dram_tensor("dxh", [ntok, P, kp // 2, P], U16,
                               kind="ExternalOutput").ap()
        dxl_d = nc.dram_tensor("dxl", [ntok, P, kp // 2, P], U16,
                               kind="ExternalOutput").ap()

    with tile.TileContext(nc) as tc, ExitStack() as ctx:
        const = ctx.enter_context(tc.tile_pool(name="const", bufs=1))
        wtp = ctx.enter_context(tc.tile_pool(name="wt", bufs=1))
        wstg = ctx.enter_context(tc.tile_pool(name="wstg", bufs=WSTG_BUFS))
        wqp = ctx.enter_context(tc.tile_pool(name="wqp", bufs=2))
        wbp = ctx.enter_context(tc.tile_pool(name="wbp", bufs=WB_BUFS))
        wtmp = ctx.enter_context(tc.tile_pool(name="wtmp", bufs=2))
        x32p = ctx.enter_context(tc.tile_pool(name="x32", bufs=2))
        hip = ctx.enter_context(tc.tile_pool(name="hi8", bufs=HI_BUFS))
        lop = ctx.enter_context(tc.tile_pool(name="lo8", bufs=LO_BUFS))
        xthp = ctx.enter_context(tc.tile_pool(name="xth", bufs=XT_BUFS))
        xtlp = ctx.enter_context(tc.tile_pool(name="xtl", bufs=XTL_BUFS))
        otp = ctx.enter_context(tc.tile_pool(name="ot16", bufs=2))
        ytp = ctx.enter_context(tc.tile_pool(name="yt16", bufs=1))
        y32p = ctx.enter_context(tc.tile_pool(name="y32", bufs=2))
        pout = ctx.enter_context(tc.tile_pool(name="pout", bufs=2, space="PSUM"))

        # scale/bias arranged (partition, chunk): [p, c] = value[128c + p];
        # pad partitions of the last chunk are zeroed so every chunk drains a
        # full 128 partitions from initialized memory
        scale_sb = const.tile([P, nfc], F32)
        bias_sb = const.tile([P, nfc], F32)
        lastsz = feat - (nfc - 1) * P
        for src, dst in ((sc_d, scale_sb), (bi_d, bias_sb)):
            if lastsz < P:
                nc.vector.memset(dst[lastsz:, nfc - 1:nfc], 0.0)
            nc.sync.dma_start(
                out=dst[:, :nfc - 1],
                in_=src[:(nfc - 1) * P].rearrange("(c p) -> p c", p=P))
            nc.sync.dma_start(
                out=dst[:lastsz, nfc - 1:nfc],
                in_=src[(nfc - 1) * P:].rearrange("(c p) -> p c", c=1))

        # Stationary weights, pair-contiguous: [k-pair(part), kblk, i, feat]
        # where element (p, j, i, f) = w[f, 256j + 2p + i]. The feat axis is
        # padded to nfc*128 with zero weights so every matmul/drain covers a
        # full 128 output partitions (cost is per output ROW, so this is free)
        feat_pad = nfc * P
        wT8x = wtp.tile([P, kstep, 2, feat_pad], F8)
        if feat_pad > feat:
            nc.gpsimd.memset(wT8x[:, :, :, feat:], 0.0)

        # ---- Phase W: unpack -> fp8 -> u16-pair transpose -> deinterleave ----
        # Nibbles sign-extended in 2 fused ALU ops ((q<<28)>>a28 / (q<<24)>>a28)
        # with the int32 ALU result converted to fp8 by the strided write.
        wq_tiles = {}

        def emit_wload(ftidx, f0, fsz):
            wqt = wqp.tile([P, half], I32)
            ldq = (nc.sync, nc.scalar, nc.gpsimd)[ftidx % 3]
            ldq.dma_start(out=wqt[:fsz], in_=wq_d[f0:f0 + fsz])
            wq_tiles[ftidx] = wqt

        wphase = {}

        def emit_wtile(ftidx, f0, fsz):
            wqt = wq_tiles.pop(ftidx)
            wb8 = wbp.tile([P, in_dim], F8)
            wb8v = wb8[:fsz].rearrange("p (i two) -> p two i", two=2)
            # bitVec TSP ops cannot cast and only run on DVE; bias the nibbles
            # in int32 there, then convert+subtract-8 on ACT (Copy, float bias)
            n_lo = wtmp.tile([P, half], I32)
            nc.vector.tensor_scalar(
                out=n_lo[:fsz], in0=wqt[:fsz], scalar1=15, scalar2=8,
                op0=mybir.AluOpType.bitwise_and, op1=mybir.AluOpType.bitwise_xor)
            n_hi = wtmp.tile([P, half], I32)
            nc.vector.tensor_scalar(
                out=n_hi[:fsz], in0=wqt[:fsz], scalar1=4, scalar2=8,
                op0=mybir.AluOpType.logical_shift_right,
                op1=mybir.AluOpType.bitwise_xor)
            c0 = nc.scalar.activation(out=wb8v[:, 0], in_=n_lo[:fsz],
                                      func=mybir.ActivationFunctionType.Copy,
                                      bias=-8.0)
            c1 = nc.scalar.activation(out=wb8v[:, 1], in_=n_hi[:fsz],
                                      func=mybir.ActivationFunctionType.Copy,
                                      bias=-8.0)
            if ftidx >= WB_BUFS:  # WAR vs the bitcast transpose read
                _dep(c0, wphase[ftidx - WB_BUFS]["wtr"],
                     reason="wb8 buffer WAR vs bitcast read")
                _dep(c1, wphase[ftidx - WB_BUFS]["wtr"],
                     reason="wb8 buffer WAR vs bitcast read")
            stg = wstg.tile([P, kstep, P], U16)
            # all DmaTransposeAnt share the ACT queue: concurrent transposes
            # on different hwdge queues corrupt each other (shared xbar)
            wtr = nc.scalar.dma_start_transpose(out=stg[:, :, :fsz],
                                                in_=wb8[:fsz].bitcast(U16))
            _dep(wtr, c0, reason="w transpose reads wb8 bitcast")
            _dep(wtr, c1, reason="w transpose reads wb8 bitcast")
            if ftidx >= WSTG_BUFS:  # WAR vs the bitcast deint reads
                for d in wphase[ftidx - WSTG_BUFS]["deints"]:
                    _dep(wtr, d, reason="stg buffer WAR vs bitcast read")
            # deinterleave the (k, k+1) byte pairs into the stationary layout
            sv = stg[:].rearrange("p a b -> p (a b)").bitcast(F8).rearrange(
                "p (j f two) -> p j two f", j=kstep, two=2)
            deints = []
            for b in range(2):
                d = nc.gpsimd.tensor_copy(out=wT8x[:, :, b, f0:f0 + fsz],
                                          in_=sv[:, :, b, :fsz])
                _dep(d, wtr, reason="deint reads stg bitcast")
                deints.append(d)
            wphase[ftidx] = {"wtr": wtr, "deints": deints}

        # All DmaTransposeAnt instructions are serialized through a global
        # dep chain so no two ever overlap (shared-xbar corruption); this
        # lets the back-transpose ride the otherwise idle SP queue.
        tchain = {"last": None}

        def _chain_transpose(t):
            if tchain["last"] is not None:
                _dep(t, tchain["last"], reason="xbar serialization chain")
            tchain["last"] = t

        # ---- Main loop stages ----
        # The dependency tracker does not see accesses made through bitcast
        # views, so every such read/write gets an explicit add_dep_helper edge
        # (RAW: view-reader after producer; WAR: buffer re-writer after the
        # last view-reader).
        state = {}

        def emit_load(i):
            x32 = x32p.tile([P, in_dim], F32)
            nc.sync.dma_start(out=x32[:], in_=x_d[i * P:(i + 1) * P])
            state[i] = {"x32": x32}

        def emit_convert(i):
            st = state[i]
            x32 = st["x32"]
            hi8 = hip.tile([P, in_dim], F8)
            cvt = nc.scalar.activation(out=hi8[:], in_=x32[:],
                                       func=mybir.ActivationFunctionType.Copy)
            on_dve = False
            if i >= HI_BUFS:
                _dep(cvt, state[i - HI_BUFS]["thi"],
                     reason="hi8 buffer WAR vs bitcast T_hi read")
            st["hi8"], st["cvt"] = hi8, cvt
            if not on_dve:
                # T_hi issued on ACT right after the ACT cvt (same engine, no
                # sequencer stall); RAW through the bitcast view
                emit_thi(i)
            lo8 = lop.tile([P, in_dim], F8)
            sub = nc.vector.tensor_tensor(out=lo8[:], in0=x32[:], in1=hi8[:],
                                          op=mybir.AluOpType.subtract)
            if i >= LO_BUFS:
                _dep(sub, state[i - LO_BUFS]["tlo"],
                     reason="lo8 buffer WAR vs bitcast T_lo read")
            st["lo8"], st["sub"] = lo8, sub

        def emit_thi(i):
            st = state[i]
            xth = xthp.tile([P, kstep, P], U16)
            thi = nc.scalar.dma_start_transpose(out=xth[:],
                                                in_=st["hi8"][:].bitcast(U16))
            _dep(thi, st["cvt"], reason="T_hi reads hi8 via bitcast")
            if i >= XT_BUFS:
                _dep(thi, state[i - XT_BUFS]["mm_last"],
                     reason="xth buffer WAR vs bitcast readers")
            if debug_xt:
                nc.gpsimd.dma_start(out=dxh_d[i], in_=xth[:])
            st["xth"] = xth
            st["thi"] = thi

        def emit_tlo(i):
            # lo transpose one iteration later: lo8(i) is long done, so the
            # ACT sequencer never blocks on the DVE semaphore
            st = state[i]
            if "xth" not in st:
                emit_thi(i)  # DVE-converted tiles get their T_hi here too
            xtl = xtlp.tile([P, kstep - LO_SKIP, P], U16)
            tlo = nc.scalar.dma_start_transpose(
                out=xtl[:], in_=st["lo8"][:, LO_SKIP * 256:].bitcast(U16))
            _dep(tlo, st["sub"], reason="T_lo reads lo8 via bitcast")
            if i >= XTL_BUFS:
                _dep(tlo, state[i - XTL_BUFS]["mm_last"],
                     reason="xtl buffer WAR vs bitcast readers")
            if debug_xt:
                nc.gpsimd.dma_start(out=dxl_d[i], in_=xtl[:])
            st["xtl"] = xtl
            st["tlo"] = tlo

        def emit_mm(i, po):
            st = state[i]
            st["po"] = po
            vh = st["xth"][:].rearrange("p a b -> p (a b)").bitcast(F8).rearrange(
                "p (j t two) -> p j two t", j=kstep, two=2)
            vl = st["xtl"][:].rearrange("p a b -> p (a b)").bitcast(F8).rearrange(
                "p (j t two) -> p j two t", j=kstep - LO_SKIP, two=2)
            first = None
            last = None
            for c, f0, fsz in fchunks:
                for j in range(kstep):
                    for b, v in ((0, vh), (1, vl)):
                        if b == 1 and j < LO_SKIP:
                            continue
                        last = nc.tensor.matmul(
                            out=po[:, c, :],
                            lhsT=wT8x[:, j, :, f0:f0 + P],
                            rhs=v[:, j - (LO_SKIP if b else 0), :, :],
                            start=(j == 0 and b == 0),
                            stop=(j == kstep - 1 and b == 1),
                            perf_mode=mybir.MatmulPerfMode.DoubleRow)
                        if first is None:
                            first = last
            # RAW: matmuls read xth/xtl via bitcast views (PE is in-order, so
            # an edge on the first matmul covers the whole tile)
            _dep(first, st["thi"], reason="mm reads xth bitcast")
            _dep(first, st["tlo"], reason="mm reads xtl bitcast")
            st["mm_last"] = last

        def emit_drainblock(i):
            # one iteration after the matmuls: all chunk stops are long past,
            # so these head-of-stream DVE ops run immediately and release the
            # PSUM buffer well before its next writer needs it
            st = state[i]
            po = st["po"]
            ot16 = otp.tile([P, nfc, P], F16)
            st["ot16"] = ot16
            for c, f0, fsz in fchunks:
                nc.vector.tensor_scalar(
                    out=ot16[:, c, :], in0=po[:, c, :],
                    scalar1=scale_sb[:, c:c + 1],
                    scalar2=bias_sb[:, c:c + 1],
                    op0=mybir.AluOpType.mult, op1=mybir.AluOpType.add)

        def emit_ytail(i):
            # one iteration after the drain: ot16(i) is complete, so the SP
            # sequencer never blocks waiting on DVE before issuing the x load
            ot16 = state[i]["ot16"]
            yt16 = ytp.tile([P, nfc, P], F16)
            nc.scalar.dma_start_transpose(
                out=yt16[:], in_=ot16[:].rearrange("p a b -> p (a b)"))
            y32 = y32p.tile([P, feat], F32)
            nc.gpsimd.tensor_copy(
                out=y32[:], in_=yt16[:].rearrange("p a b -> p (a b)")[:, :feat])
            nc.gpsimd.dma_start(out=y_d[i * P:(i + 1) * P, :], in_=y32[:])

        wdims = [(c * P, min(P, feat - c * P)) for c in range(nfc)]
        for ftidx in range(nfc + 2):
            if ftidx < nfc:
                emit_wload(ftidx, *wdims[ftidx])
            if ftidx >= 2:
                emit_wtile(ftidx - 2, *wdims[ftidx - 2])

        for i in range(ntok + 6):
            if 4 <= i <= ntok + 3:
                emit_drainblock(i - 4)
            if 5 <= i <= ntok + 4:
                emit_ytail(i - 5)
            if i < ntok:
                emit_load(i)
            if 1 <= i <= ntok:
                emit_convert(i - 1)
            if 2 <= i <= ntok + 1:
                emit_tlo(i - 2)
            if 3 <= i <= ntok + 2:
                po = pout.tile([P, nfc, P], F32)
                emit_mm(i - 3, po)
            if i >= 8:
                del state[i - 8]

    nc.compile()
    return nc


_CACHE = {}


def _get_program():
    if "nc" not in _CACHE:
        _CACHE["nc"] = build()
    return _CACHE["nc"]


def kernel(x, weight_q, scale, bias):
    from concourse.bass_utils import run_bass_kernel_spmd

    try:
        import jax

        jax.config.update("jax_compilation_cache_dir", "/root/problem/jax_cache")
        jax.config.update("jax_persistent_cache_min_compile_time_secs", 0)
    except Exception:
        pass

    nc = _get_program()
    xr = np.ascontiguousarray(np.asarray(x, dtype=np.float32).reshape(TOK, IN))
    wq = np.asarray(weight_q, dtype=np.int32)
    sc = np.asarray(scale, dtype=np.float32)
    bi = np.asarray(bias, dtype=np.float32)
    in_maps = []
    for c in range(NCORES):
        f0 = c * FEAT
        in_maps.append({
            "x": xr,
            "wq": np.ascontiguousarray(wq[f0:f0 + FEAT]),
            "scale": np.ascontiguousarray(sc[f0:f0 + FEAT]),
            "bias": np.ascontiguousarray(bi[f0:f0 + FEAT]),
        })
    res = run_bass_kernel_spmd(nc, in_maps, list(range(NCORES))).results
    y = np.concatenate([res[c]["y"] for c in range(NCORES)], axis=1)
    return y.reshape(B, S, OUT)


# revision 90
# speedup vs baseline: 1.5443x; 1.0834x over previous
"""Int4-quantized column-parallel linear (LLaMA-7B FFN up-proj) on 8 TRN2 cores.

y[b,s,o] = sum_i x[b,s,i] * (unpack_int4(weight_q)[o,i] * scale[o]) + bias[o]

Strategy (per core, 1/8 of out_features = 1376):
  - int4 weight nibbles are exact in fp8 e4m3; x is split x ~= hi + lo with
    hi = fp8(x), lo = fp8(x - hi), so the pair of fp8 matmuls reconstructs x
    to ~8e-4 relative. The lo-correction is skipped for LO_SKIP of the 16
    k-steps (residual error 2.65%*sqrt(LO_SKIP/16), far inside the 2e-2 gate),
    saving 1/32 of PE cycles per skipped step.
  - all matmuls run in MatmulPerfMode.DoubleRow (fp8 double-pumping): one
    instruction contracts 2x128 K rows at 0.5 PE cycles per output row.
  - layout: hi8/lo8 are converted straight from f32 x (no fp16 step), then
    DMA-transposed as uint16 byte-pairs; the resulting pair-interleaved
    [k-pair, 2, tok] operand rides the MOVING side of the matmul (the ISA
    allows strided moving APs but not strided LDWEIGHTS), with the weights
    pre-deinterleaved once into the matching pair-contiguous stationary
    layout. Output lands PSUM-[feat, tok], is drained with a fused per-
    partition scale*psum+bias tensor_scalar to fp16, DMA-transposed back to
    [tok, feat], and converted to f32.
"""

from contextlib import ExitStack

import numpy as np

import concourse.bass as bass
import concourse.tile as tile
from concourse import bacc, mybir

F32 = mybir.dt.float32
F16 = mybir.dt.float16
F8 = mybir.dt.float8e4
I32 = mybir.dt.int32
U16 = mybir.dt.uint16
U8 = mybir.dt.uint8

B, S, IN, OUT = 4, 2048, 4096, 11008
NCORES = 8
TOK = B * S
FEAT = OUT // NCORES

P = 128
KP = IN // P          # 32 k-subtiles of 128
KSTEP = KP // 2       # 16 DoubleRow k-steps of 256
NTOK = TOK // P       # 64 token tiles
HALF = IN // 2

# Number of 256-row k-steps whose lo-correction matmul is skipped (of KSTEP).
LO_SKIP = 3

# Pool depths for the x pipeline (referenced by the manual WAR dep edges).
HI_BUFS = 2
LO_BUFS = 4
XT_BUFS = 6
XTL_BUFS = 4
WB_BUFS = 2
WSTG_BUFS = 1


def _dep(from_inst, to_inst, reason):
    tile.add_dep_helper(from_inst.ins, to_inst.ins, reason=reason)


def build(tok=TOK, in_dim=IN, feat=FEAT, debug_xt=False):
    assert tok % P == 0 and in_dim % 256 == 0
    ntok = tok // P
    kp = in_dim // P
    kstep = kp // 2
    half = in_dim // 2
    fchunks = [(c, c * P, min(P, feat - c * P)) for c in range((feat + P - 1) // P)]
    nfc = len(fchunks)

    nc = bacc.Bacc("TRN2", target_bir_lowering=False, debug=False,
                   num_devices=NCORES)
    x_d = nc.dram_tensor("x", [tok, in_dim], F32, kind="ExternalInput").ap()
    wq_d = nc.dram_tensor("wq", [feat, half], U8, kind="ExternalInput").ap()
    sc_d = nc.dram_tensor("scale", [feat], F32, kind="ExternalInput").ap()
    bi_d = nc.dram_tensor("bias", [feat], F32, kind="ExternalInput").ap()
    nfc_ = (feat + P - 1) // P
    y_d = nc.dram_tensor("y", [nfc_ * P, tok], F32, kind="ExternalOutput").ap()
    if debug_xt:
        dxh_d = nc.dram_tensor("dxh", [ntok, P, kp // 2, P], U16,
                               kind="ExternalOutput").ap()
        dxl_d = nc.dram_tensor("dxl", [ntok, P, kp // 2, P], U16,
                               kind="ExternalOutput").ap()

    with tile.TileContext(nc) as tc, ExitStack() as ctx:
        const = ctx.enter_context(tc.tile_pool(name="const", bufs=1))
        wtp = ctx.enter_context(tc.tile_pool(name="wt", bufs=1))
        wstg = ctx.enter_context(tc.tile_pool(name="wstg", bufs=WSTG_BUFS))
        wqp = ctx.enter_context(tc.tile_pool(name="wqp", bufs=2))
        wbp = ctx.enter_context(tc.tile_pool(name="wbp", bufs=WB_BUFS))
        wtmp = ctx.enter_context(tc.tile_pool(name="wtmp", bufs=2))
        x32p = ctx.enter_context(tc.tile_pool(name="x32", bufs=2))
        hip = ctx.enter_context(tc.tile_pool(name="hi8", bufs=HI_BUFS))
        lop = ctx.enter_context(tc.tile_pool(name="lo8", bufs=LO_BUFS))
        xthp = ctx.enter_context(tc.tile_pool(name="xth", bufs=XT_BUFS))
        xtlp = ctx.enter_context(tc.tile_pool(name="xtl", bufs=XTL_BUFS))
        otp = ctx.enter_context(tc.tile_pool(name="ot32", bufs=3))
        pout = ctx.enter_context(tc.tile_pool(name="pout", bufs=2, space="PSUM"))

        # scale/bias arranged (partition, chunk): [p, c] = value[128c + p];
        # pad partitions of the last chunk are zeroed so every chunk drains a
        # full 128 partitions from initialized memory
        scale_sb = const.tile([P, nfc], F32)
        bias_sb = const.tile([P, nfc], F32)
        lastsz = feat - (nfc - 1) * P
        for src, dst in ((sc_d, scale_sb), (bi_d, bias_sb)):
            if lastsz < P:
                nc.vector.memset(dst[lastsz:, nfc - 1:nfc], 0.0)
            nc.sync.dma_start(
                out=dst[:, :nfc - 1],
                in_=src[:(nfc - 1) * P].rearrange("(c p) -> p c", p=P))
            nc.sync.dma_start(
                out=dst[:lastsz, nfc - 1:nfc],
                in_=src[(nfc - 1) * P:].rearrange("(c p) -> p c", c=1))

        # Stationary weights, pair-contiguous: [k-pair(part), kblk, i, feat]
        # where element (p, j, i, f) = w[f, 256j + 2p + i]. The feat axis is
        # padded to nfc*128 with zero weights so every matmul/drain covers a
        # full 128 output partitions (cost is per output ROW, so this is free)
        feat_pad = nfc * P
        wT8x = wtp.tile([P, kstep, 2, feat_pad], F8)
        if feat_pad > feat:
            nc.gpsimd.memset(wT8x[:, :, :, feat:], 0.0)

        # ---- Phase W: unpack -> fp8 -> u16-pair transpose -> deinterleave ----
        # Nibbles sign-extended in 2 fused ALU ops ((q<<28)>>a28 / (q<<24)>>a28)
        # with the int32 ALU result converted to fp8 by the strided write.
        wq_tiles = {}

        def emit_wload(ftidx, f0, fsz):
            wqt = wqp.tile([P, half], U8)
            ldq = (nc.sync, nc.scalar, nc.gpsimd)[ftidx % 3]
            ldq.dma_start(out=wqt[:fsz], in_=wq_d[f0:f0 + fsz])
            wq_tiles[ftidx] = wqt

        wphase = {}

        def emit_wtile(ftidx, f0, fsz):
            wqt = wq_tiles.pop(ftidx)
            wb8 = wbp.tile([P, in_dim], F8)
            wb8v = wb8[:fsz].rearrange("p (i two) -> p two i", two=2)
            # bitVec TSP ops cannot cast and only run on DVE; bias the nibbles
            # in int32 there, then convert+subtract-8 on ACT (Copy, float bias)
            n_lo = wtmp.tile([P, half], U8)
            nc.vector.tensor_scalar(
                out=n_lo[:fsz], in0=wqt[:fsz], scalar1=15, scalar2=8,
                op0=mybir.AluOpType.bitwise_and, op1=mybir.AluOpType.bitwise_xor)
            n_hi = wtmp.tile([P, half], U8)
            nc.vector.tensor_scalar(
                out=n_hi[:fsz], in0=wqt[:fsz], scalar1=4, scalar2=8,
                op0=mybir.AluOpType.logical_shift_right,
                op1=mybir.AluOpType.bitwise_xor)
            c0 = nc.scalar.activation(out=wb8v[:, 0], in_=n_lo[:fsz],
                                      func=mybir.ActivationFunctionType.Copy,
                                      bias=-8.0)
            c1 = nc.scalar.activation(out=wb8v[:, 1], in_=n_hi[:fsz],
                                      func=mybir.ActivationFunctionType.Copy,
                                      bias=-8.0)
            if ftidx >= WB_BUFS:  # WAR vs the bitcast transpose read
                _dep(c0, wphase[ftidx - WB_BUFS]["wtr"],
                     reason="wb8 buffer WAR vs bitcast read")
                _dep(c1, wphase[ftidx - WB_BUFS]["wtr"],
                     reason="wb8 buffer WAR vs bitcast read")
            stg = wstg.tile([P, kstep, P], U16)
            # all DmaTransposeAnt share the ACT queue: concurrent transposes
            # on different hwdge queues corrupt each other (shared xbar)
            wtr = nc.scalar.dma_start_transpose(out=stg[:, :, :fsz],
                                                in_=wb8[:fsz].bitcast(U16))
            _dep(wtr, c0, reason="w transpose reads wb8 bitcast")
            _dep(wtr, c1, reason="w transpose reads wb8 bitcast")
            if ftidx >= WSTG_BUFS:  # WAR vs the bitcast deint reads
                for d in wphase[ftidx - WSTG_BUFS]["deints"]:
                    _dep(wtr, d, reason="stg buffer WAR vs bitcast read")
            # deinterleave the (k, k+1) byte pairs into the stationary layout
            sv = stg[:].rearrange("p a b -> p (a b)").bitcast(F8).rearrange(
                "p (j f two) -> p j two f", j=kstep, two=2)
            deints = []
            for b in range(2):
                d = nc.gpsimd.tensor_copy(out=wT8x[:, :, b, f0:f0 + fsz],
                                          in_=sv[:, :, b, :fsz])
                _dep(d, wtr, reason="deint reads stg bitcast")
                deints.append(d)
            wphase[ftidx] = {"wtr": wtr, "deints": deints}

        # All DmaTransposeAnt instructions are serialized through a global
        # dep chain so no two ever overlap (shared-xbar corruption); this
        # lets the back-transpose ride the otherwise idle SP queue.
        tchain = {"last": None}

        def _chain_transpose(t):
            if tchain["last"] is not None:
                _dep(t, tchain["last"], reason="xbar serialization chain")
            tchain["last"] = t

        # ---- Main loop stages ----
        # The dependency tracker does not see accesses made through bitcast
        # views, so every such read/write gets an explicit add_dep_helper edge
        # (RAW: view-reader after producer; WAR: buffer re-writer after the
        # last view-reader).
        state = {}

        def emit_load(i):
            x32 = x32p.tile([P, in_dim], F32)
            nc.sync.dma_start(out=x32[:], in_=x_d[i * P:(i + 1) * P])
            state[i] = {"x32": x32}

        def emit_convert(i):
            st = state[i]
            x32 = st["x32"]
            hi8 = hip.tile([P, in_dim], F8)
            cvt = nc.scalar.activation(out=hi8[:], in_=x32[:],
                                       func=mybir.ActivationFunctionType.Copy)
            on_dve = False
            if i >= HI_BUFS:
                _dep(cvt, state[i - HI_BUFS]["thi"],
                     reason="hi8 buffer WAR vs bitcast T_hi read")
            st["hi8"], st["cvt"] = hi8, cvt
            if not on_dve:
                # T_hi issued on ACT right after the ACT cvt (same engine, no
                # sequencer stall); RAW through the bitcast view
                emit_thi(i)
            lo8 = lop.tile([P, in_dim], F8)
            sub = nc.vector.tensor_tensor(out=lo8[:], in0=x32[:], in1=hi8[:],
                                          op=mybir.AluOpType.subtract)
            if i >= LO_BUFS:
                _dep(sub, state[i - LO_BUFS]["tlo"],
                     reason="lo8 buffer WAR vs bitcast T_lo read")
            st["lo8"], st["sub"] = lo8, sub

        def emit_thi(i):
            st = state[i]
            xth = xthp.tile([P, kstep, P], U16)
            thi = nc.scalar.dma_start_transpose(out=xth[:],
                                                in_=st["hi8"][:].bitcast(U16))
            _dep(thi, st["cvt"], reason="T_hi reads hi8 via bitcast")
            if i >= XT_BUFS:
                _dep(thi, state[i - XT_BUFS]["mm_last"],
                     reason="xth buffer WAR vs bitcast readers")
            if debug_xt:
                nc.gpsimd.dma_start(out=dxh_d[i], in_=xth[:])
            st["xth"] = xth
            st["thi"] = thi

        def emit_tlo(i):
            # lo transpose one iteration later: lo8(i) is long done, so the
            # ACT sequencer never blocks on the DVE semaphore
            st = state[i]
            if "xth" not in st:
                emit_thi(i)  # DVE-converted tiles get their T_hi here too
            xtl = xtlp.tile([P, kstep - LO_SKIP, P], U16)
            tlo = nc.scalar.dma_start_transpose(
                out=xtl[:], in_=st["lo8"][:, LO_SKIP * 256:].bitcast(U16))
            _dep(tlo, st["sub"], reason="T_lo reads lo8 via bitcast")
            if i >= XTL_BUFS:
                _dep(tlo, state[i - XTL_BUFS]["mm_last"],
                     reason="xtl buffer WAR vs bitcast readers")
            if debug_xt:
                nc.gpsimd.dma_start(out=dxl_d[i], in_=xtl[:])
            st["xtl"] = xtl
            st["tlo"] = tlo

        def emit_mm(i, po):
            st = state[i]
            st["po"] = po
            vh = st["xth"][:].rearrange("p a b -> p (a b)").bitcast(F8).rearrange(
                "p (j t two) -> p j two t", j=kstep, two=2)
            vl = st["xtl"][:].rearrange("p a b -> p (a b)").bitcast(F8).rearrange(
                "p (j t two) -> p j two t", j=kstep - LO_SKIP, two=2)
            first = None
            last = None
            for c, f0, fsz in fchunks:
                for j in range(kstep):
                    for b, v in ((0, vh), (1, vl)):
                        if b == 1 and j < LO_SKIP:
                            continue
                        last = nc.tensor.matmul(
                            out=po[:, c, :],
                            lhsT=wT8x[:, j, :, f0:f0 + P],
                            rhs=v[:, j - (LO_SKIP if b else 0), :, :],
                            start=(j == 0 and b == 0),
                            stop=(j == kstep - 1 and b == 1),
                            perf_mode=mybir.MatmulPerfMode.DoubleRow)
                        if first is None:
                            first = last
            # RAW: matmuls read xth/xtl via bitcast views (PE is in-order, so
            # an edge on the first matmul covers the whole tile)
            _dep(first, st["thi"], reason="mm reads xth bitcast")
            _dep(first, st["tlo"], reason="mm reads xtl bitcast")
            st["mm_last"] = last

        def emit_drainblock(i):
            # one iteration after the matmuls: all chunk stops are long past,
            # so these head-of-stream DVE ops run immediately and release the
            # PSUM buffer well before its next writer needs it
            st = state[i]
            po = st["po"]
            ot32 = otp.tile([P, nfc, P], F32)
            st["ot32"] = ot32
            for c, f0, fsz in fchunks:
                nc.vector.tensor_scalar(
                    out=ot32[:, c, :], in0=po[:, c, :],
                    scalar1=scale_sb[:, c:c + 1],
                    scalar2=bias_sb[:, c:c + 1],
                    op0=mybir.AluOpType.mult, op1=mybir.AluOpType.add)

        def emit_ytail(i):
            # output stays [feat, tok]-oriented; the host unshard transposes.
            # One strided DMA per tile: runs of 128 f32 (512B descriptors)
            ot32 = state[i]["ot32"]
            dst = y_d[:, i * P:(i + 1) * P].rearrange("(c p) t -> p c t", p=P)
            nc.gpsimd.dma_start(out=dst, in_=ot32[:])

        wdims = [(c * P, min(P, feat - c * P)) for c in range(nfc)]
        for ftidx in range(nfc + 2):
            if ftidx < nfc:
                emit_wload(ftidx, *wdims[ftidx])
            if ftidx >= 2:
                emit_wtile(ftidx - 2, *wdims[ftidx - 2])

        for i in range(ntok + 6):
            if 4 <= i <= ntok + 3:
                emit_drainblock(i - 4)
            if 5 <= i <= ntok + 4:
                emit_ytail(i - 5)
            if i < ntok:
                emit_load(i)
            if 1 <= i <= ntok:
                emit_convert(i - 1)
            if 2 <= i <= ntok + 1:
                emit_tlo(i - 2)
            if 3 <= i <= ntok + 2:
                po = pout.tile([P, nfc, P], F32)
                emit_mm(i - 3, po)
            if i >= 8:
                del state[i - 8]

    nc.compile()
    return nc


_CACHE = {}


def _get_program():
    if "nc" not in _CACHE:
        _CACHE["nc"] = build()
    return _CACHE["nc"]


def kernel(x, weight_q, scale, bias):
    from concourse.bass_utils import run_bass_kernel_spmd

    try:
        import jax

        jax.config.update("jax_compilation_cache_dir", "/root/problem/jax_cache")
        jax.config.update("jax_persistent_cache_min_compile_time_secs", 0)
    except Exception:
        pass

    nc = _get_program()
    xr = np.ascontiguousarray(np.asarray(x, dtype=np.float32).reshape(TOK, IN))
    wq = np.asarray(weight_q, dtype=np.int32)
    sc = np.asarray(scale, dtype=np.float32)
    bi = np.asarray(bias, dtype=np.float32)
    in_maps = []
    for c in range(NCORES):
        f0 = c * FEAT
        in_maps.append({
            "x": xr,
            "wq": np.ascontiguousarray(wq[f0:f0 + FEAT].astype(np.uint8)),
            "scale": np.ascontiguousarray(sc[f0:f0 + FEAT]),
            "bias": np.ascontiguousarray(bi[f0:f0 + FEAT]),
        })
    res = run_bass_kernel_spmd(nc, in_maps, list(range(NCORES))).results
    y = np.concatenate([res[c]["y"][:FEAT].T for c in range(NCORES)], axis=1)
    return np.ascontiguousarray(y).reshape(B, S, OUT)


# revision 100
# speedup vs baseline: 1.5625x; 1.0118x over previous
"""Int4-quantized column-parallel linear (LLaMA-7B FFN up-proj) on 8 TRN2 cores.

y[b,s,o] = sum_i x[b,s,i] * (unpack_int4(weight_q)[o,i] * scale[o]) + bias[o]

Strategy (per core, 1/8 of out_features = 1376):
  - int4 weight nibbles are exact in fp8 e4m3; x is split x ~= hi + lo with
    hi = fp8(x), lo = fp8(x - hi), so the pair of fp8 matmuls reconstructs x
    to ~8e-4 relative. The lo-correction is skipped for LO_SKIP of the 16
    k-steps (residual error 2.65%*sqrt(LO_SKIP/16), far inside the 2e-2 gate),
    saving 1/32 of PE cycles per skipped step.
  - all matmuls run in MatmulPerfMode.DoubleRow (fp8 double-pumping): one
    instruction contracts 2x128 K rows at 0.5 PE cycles per output row.
  - layout: hi8/lo8 are converted straight from f32 x (no fp16 step), then
    DMA-transposed as uint16 byte-pairs; the resulting pair-interleaved
    [k-pair, 2, tok] operand rides the MOVING side of the matmul (the ISA
    allows strided moving APs but not strided LDWEIGHTS), with the weights
    pre-deinterleaved once into the matching pair-contiguous stationary
    layout. Output lands PSUM-[feat, tok], is drained with a fused per-
    partition scale*psum+bias tensor_scalar to f32 and DMA'd out still
    feat-major; the host-side unshard transposes to [tok, feat].
  - weight_q is shipped to the cores as uint8 (lossless re-encode of the
    packed byte values) to shrink the weight DMA and staging pools 4x.
  - all DMA transposes share the ACT hwdge queue (concurrent transposes on
    different queues corrupt each other on silicon), and every access made
    through a bitcast view carries an explicit dependency edge because the
    tile framework's tracker does not see them.
"""

from contextlib import ExitStack

import numpy as np

import concourse.bass as bass
import concourse.tile as tile
from concourse import bacc, mybir

F32 = mybir.dt.float32
F16 = mybir.dt.float16
F8 = mybir.dt.float8e4
I32 = mybir.dt.int32
U16 = mybir.dt.uint16
U8 = mybir.dt.uint8

B, S, IN, OUT = 4, 2048, 4096, 11008
NCORES = 8
TOK = B * S
FEAT = OUT // NCORES

P = 128
KP = IN // P          # 32 k-subtiles of 128
KSTEP = KP // 2       # 16 DoubleRow k-steps of 256
NTOK = TOK // P       # 64 token tiles
HALF = IN // 2

# Number of 256-row k-steps whose lo-correction matmul is skipped (of KSTEP).
LO_SKIP = 3

# Pool depths for the x pipeline (referenced by the manual WAR dep edges).
HI_BUFS = 2
LO_BUFS = 4
XT_BUFS = 6
XTL_BUFS = 4
WB_BUFS = 2
WSTG_BUFS = 1


def _dep(from_inst, to_inst, reason):
    tile.add_dep_helper(from_inst.ins, to_inst.ins, reason=reason)


def build(tok=TOK, in_dim=IN, feat=FEAT, debug_xt=False):
    assert tok % P == 0 and in_dim % 256 == 0
    ntok = tok // P
    kp = in_dim // P
    kstep = kp // 2
    half = in_dim // 2
    fchunks = [(c, c * P, min(P, feat - c * P)) for c in range((feat + P - 1) // P)]
    nfc = len(fchunks)

    nc = bacc.Bacc("TRN2", target_bir_lowering=False, debug=False,
                   num_devices=NCORES)
    x_d = nc.dram_tensor("x", [tok, in_dim], F32, kind="ExternalInput").ap()
    wq_d = nc.dram_tensor("wq", [feat, half], U8, kind="ExternalInput").ap()
    sc_d = nc.dram_tensor("scale", [feat], F32, kind="ExternalInput").ap()
    bi_d = nc.dram_tensor("bias", [feat], F32, kind="ExternalInput").ap()
    nfc_ = (feat + P - 1) // P
    y_d = nc.dram_tensor("y", [nfc_ * P, tok], F32, kind="ExternalOutput").ap()
    if debug_xt:
        dxh_d = nc.dram_tensor("dxh", [ntok, P, kp // 2, P], U16,
                               kind="ExternalOutput").ap()
        dxl_d = nc.dram_tensor("dxl", [ntok, P, kp // 2, P], U16,
                               kind="ExternalOutput").ap()

    with tile.TileContext(nc) as tc, ExitStack() as ctx:
        const = ctx.enter_context(tc.tile_pool(name="const", bufs=1))
        wtp = ctx.enter_context(tc.tile_pool(name="wt", bufs=1))
        wstg = ctx.enter_context(tc.tile_pool(name="wstg", bufs=WSTG_BUFS))
        wqp = ctx.enter_context(tc.tile_pool(name="wqp", bufs=5))
        wbp = ctx.enter_context(tc.tile_pool(name="wbp", bufs=WB_BUFS))
        wtmp = ctx.enter_context(tc.tile_pool(name="wtmp", bufs=2))
        x32p = ctx.enter_context(tc.tile_pool(name="x32", bufs=2))
        hip = ctx.enter_context(tc.tile_pool(name="hi8", bufs=HI_BUFS))
        lop = ctx.enter_context(tc.tile_pool(name="lo8", bufs=LO_BUFS))
        xthp = ctx.enter_context(tc.tile_pool(name="xth", bufs=XT_BUFS))
        xtlp = ctx.enter_context(tc.tile_pool(name="xtl", bufs=XTL_BUFS))
        otp = ctx.enter_context(tc.tile_pool(name="ot32", bufs=3))
        pout = ctx.enter_context(tc.tile_pool(name="pout", bufs=2, space="PSUM"))

        # scale/bias arranged (partition, chunk): [p, c] = value[128c + p];
        # pad partitions of the last chunk are zeroed so every chunk drains a
        # full 128 partitions from initialized memory
        scale_sb = const.tile([P, nfc], F32)
        bias_sb = const.tile([P, nfc], F32)
        lastsz = feat - (nfc - 1) * P
        for src, dst in ((sc_d, scale_sb), (bi_d, bias_sb)):
            if lastsz < P:
                nc.vector.memset(dst[lastsz:, nfc - 1:nfc], 0.0)
            nc.sync.dma_start(
                out=dst[:, :nfc - 1],
                in_=src[:(nfc - 1) * P].rearrange("(c p) -> p c", p=P))
            nc.sync.dma_start(
                out=dst[:lastsz, nfc - 1:nfc],
                in_=src[(nfc - 1) * P:].rearrange("(c p) -> p c", c=1))

        # Stationary weights, pair-contiguous: [k-pair(part), kblk, i, feat]
        # where element (p, j, i, f) = w[f, 256j + 2p + i]. The feat axis is
        # padded to nfc*128 with zero weights so every matmul/drain covers a
        # full 128 output partitions (cost is per output ROW, so this is free)
        feat_pad = nfc * P
        wT8x = wtp.tile([P, kstep, 2, feat_pad], F8)
        if feat_pad > feat:
            nc.gpsimd.memset(wT8x[:, :, :, feat:], 0.0)

        # ---- Phase W: unpack -> fp8 -> u16-pair transpose -> deinterleave ----
        # Nibbles sign-extended in 2 fused ALU ops ((q<<28)>>a28 / (q<<24)>>a28)
        # with the int32 ALU result converted to fp8 by the strided write.
        wq_tiles = {}

        def emit_wload(ftidx, f0, fsz):
            # all weight loads ride the otherwise-idle Pool queue so they are
            # never stuck behind a 6us x-tile load on SP
            wqt = wqp.tile([P, half], U8)
            nc.gpsimd.dma_start(out=wqt[:fsz], in_=wq_d[f0:f0 + fsz])
            wq_tiles[ftidx] = wqt

        wphase = {}

        def emit_wtile(ftidx, f0, fsz):
            wqt = wq_tiles.pop(ftidx)
            wb8 = wbp.tile([P, in_dim], F8)
            wb8v = wb8[:fsz].rearrange("p (i two) -> p two i", two=2)
            # bitVec TSP ops cannot cast and only run on DVE; bias the nibbles
            # in int32 there, then convert+subtract-8 on ACT (Copy, float bias)
            n_lo = wtmp.tile([P, half], U8)
            nc.vector.tensor_scalar(
                out=n_lo[:fsz], in0=wqt[:fsz], scalar1=15, scalar2=8,
                op0=mybir.AluOpType.bitwise_and, op1=mybir.AluOpType.bitwise_xor)
            n_hi = wtmp.tile([P, half], U8)
            nc.vector.tensor_scalar(
                out=n_hi[:fsz], in0=wqt[:fsz], scalar1=4, scalar2=8,
                op0=mybir.AluOpType.logical_shift_right,
                op1=mybir.AluOpType.bitwise_xor)
            c0 = nc.scalar.activation(out=wb8v[:, 0], in_=n_lo[:fsz],
                                      func=mybir.ActivationFunctionType.Copy,
                                      bias=-8.0)
            c1 = nc.scalar.activation(out=wb8v[:, 1], in_=n_hi[:fsz],
                                      func=mybir.ActivationFunctionType.Copy,
                                      bias=-8.0)
            if ftidx >= WB_BUFS:  # WAR vs the bitcast transpose read
                _dep(c0, wphase[ftidx - WB_BUFS]["wtr"],
                     reason="wb8 buffer WAR vs bitcast read")
                _dep(c1, wphase[ftidx - WB_BUFS]["wtr"],
                     reason="wb8 buffer WAR vs bitcast read")
            stg = wstg.tile([P, kstep, P], U16)
            # all DmaTransposeAnt share the ACT queue: concurrent transposes
            # on different hwdge queues corrupt each other (shared xbar)
            wtr = nc.scalar.dma_start_transpose(out=stg[:, :, :fsz],
                                                in_=wb8[:fsz].bitcast(U16))
            _dep(wtr, c0, reason="w transpose reads wb8 bitcast")
            _dep(wtr, c1, reason="w transpose reads wb8 bitcast")
            if ftidx >= WSTG_BUFS:  # WAR vs the bitcast deint reads
                for d in wphase[ftidx - WSTG_BUFS]["deints"]:
                    _dep(wtr, d, reason="stg buffer WAR vs bitcast read")
            # deinterleave the (k, k+1) byte pairs into the stationary layout
            sv = stg[:].rearrange("p a b -> p (a b)").bitcast(F8).rearrange(
                "p (j f two) -> p j two f", j=kstep, two=2)
            deints = []
            for b in range(2):
                d = nc.gpsimd.tensor_copy(out=wT8x[:, :, b, f0:f0 + fsz],
                                          in_=sv[:, :, b, :fsz])
                _dep(d, wtr, reason="deint reads stg bitcast")
                deints.append(d)
            wphase[ftidx] = {"wtr": wtr, "deints": deints}

        # All DmaTransposeAnt instructions are serialized through a global
        # dep chain so no two ever overlap (shared-xbar corruption); this
        # lets the back-transpose ride the otherwise idle SP queue.
        tchain = {"last": None}

        def _chain_transpose(t):
            if tchain["last"] is not None:
                _dep(t, tchain["last"], reason="xbar serialization chain")
            tchain["last"] = t

        # ---- Main loop stages ----
        # The dependency tracker does not see accesses made through bitcast
        # views, so every such read/write gets an explicit add_dep_helper edge
        # (RAW: view-reader after producer; WAR: buffer re-writer after the
        # last view-reader).
        state = {}

        def emit_load(i):
            x32 = x32p.tile([P, in_dim], F32)
            nc.sync.dma_start(out=x32[:], in_=x_d[i * P:(i + 1) * P])
            state[i] = {"x32": x32}

        def emit_convert(i):
            st = state[i]
            x32 = st["x32"]
            hi8 = hip.tile([P, in_dim], F8)
            cvt = nc.scalar.activation(out=hi8[:], in_=x32[:],
                                       func=mybir.ActivationFunctionType.Copy)
            on_dve = False
            if i >= HI_BUFS:
                _dep(cvt, state[i - HI_BUFS]["thi"],
                     reason="hi8 buffer WAR vs bitcast T_hi read")
            st["hi8"], st["cvt"] = hi8, cvt
            if not on_dve:
                # T_hi issued on ACT right after the ACT cvt (same engine, no
                # sequencer stall); RAW through the bitcast view
                emit_thi(i)
            lo8 = lop.tile([P, in_dim], F8)
            sub = nc.vector.tensor_tensor(out=lo8[:], in0=x32[:], in1=hi8[:],
                                          op=mybir.AluOpType.subtract)
            if i >= LO_BUFS:
                _dep(sub, state[i - LO_BUFS]["tlo"],
                     reason="lo8 buffer WAR vs bitcast T_lo read")
            st["lo8"], st["sub"] = lo8, sub

        def emit_thi(i):
            st = state[i]
            xth = xthp.tile([P, kstep, P], U16)
            thi = nc.scalar.dma_start_transpose(out=xth[:],
                                                in_=st["hi8"][:].bitcast(U16))
            _dep(thi, st["cvt"], reason="T_hi reads hi8 via bitcast")
            if i >= XT_BUFS:
                _dep(thi, state[i - XT_BUFS]["mm_last"],
                     reason="xth buffer WAR vs bitcast readers")
            if debug_xt:
                nc.gpsimd.dma_start(out=dxh_d[i], in_=xth[:])
            st["xth"] = xth
            st["thi"] = thi

        def emit_tlo(i):
            # lo transpose one iteration later: lo8(i) is long done, so the
            # ACT sequencer never blocks on the DVE semaphore
            st = state[i]
            if "xth" not in st:
                emit_thi(i)  # DVE-converted tiles get their T_hi here too
            xtl = xtlp.tile([P, kstep - LO_SKIP, P], U16)
            tlo = nc.scalar.dma_start_transpose(
                out=xtl[:], in_=st["lo8"][:, LO_SKIP * 256:].bitcast(U16))
            _dep(tlo, st["sub"], reason="T_lo reads lo8 via bitcast")
            if i >= XTL_BUFS:
                _dep(tlo, state[i - XTL_BUFS]["mm_last"],
                     reason="xtl buffer WAR vs bitcast readers")
            if debug_xt:
                nc.gpsimd.dma_start(out=dxl_d[i], in_=xtl[:])
            st["xtl"] = xtl
            st["tlo"] = tlo

        def emit_mm(i, po):
            st = state[i]
            st["po"] = po
            vh = st["xth"][:].rearrange("p a b -> p (a b)").bitcast(F8).rearrange(
                "p (j t two) -> p j two t", j=kstep, two=2)
            vl = st["xtl"][:].rearrange("p a b -> p (a b)").bitcast(F8).rearrange(
                "p (j t two) -> p j two t", j=kstep - LO_SKIP, two=2)
            first = None
            last = None
            for c, f0, fsz in fchunks:
                for j in range(kstep):
                    for b, v in ((0, vh), (1, vl)):
                        if b == 1 and j < LO_SKIP:
                            continue
                        last = nc.tensor.matmul(
                            out=po[:, c, :],
                            lhsT=wT8x[:, j, :, f0:f0 + P],
                            rhs=v[:, j - (LO_SKIP if b else 0), :, :],
                            start=(j == 0 and b == 0),
                            stop=(j == kstep - 1 and b == 1),
                            perf_mode=mybir.MatmulPerfMode.DoubleRow)
                        if first is None:
                            first = last
            # RAW: matmuls read xth/xtl via bitcast views (PE is in-order, so
            # an edge on the first matmul covers the whole tile)
            _dep(first, st["thi"], reason="mm reads xth bitcast")
            _dep(first, st["tlo"], reason="mm reads xtl bitcast")
            st["mm_last"] = last

        def emit_drainblock(i):
            # one iteration after the matmuls: all chunk stops are long past,
            # so these head-of-stream DVE ops run immediately and release the
            # PSUM buffer well before its next writer needs it
            st = state[i]
            po = st["po"]
            ot32 = otp.tile([P, nfc, P], F32)
            st["ot32"] = ot32
            for c, f0, fsz in fchunks:
                nc.vector.tensor_scalar(
                    out=ot32[:, c, :], in0=po[:, c, :],
                    scalar1=scale_sb[:, c:c + 1],
                    scalar2=bias_sb[:, c:c + 1],
                    op0=mybir.AluOpType.mult, op1=mybir.AluOpType.add)

        def emit_ytail(i):
            # output stays [feat, tok]-oriented; the host unshard transposes.
            # One strided DMA per tile: runs of 128 f32 (512B descriptors)
            ot32 = state[i]["ot32"]
            dst = y_d[:, i * P:(i + 1) * P].rearrange("(c p) t -> p c t", p=P)
            nc.gpsimd.dma_start(out=dst, in_=ot32[:])

        wdims = [(c * P, min(P, feat - c * P)) for c in range(nfc)]
        for ftidx in range(nfc + 5):
            if ftidx < nfc:
                emit_wload(ftidx, *wdims[ftidx])
            if ftidx >= 5:
                emit_wtile(ftidx - 5, *wdims[ftidx - 5])

        for i in range(ntok + 6):
            if 4 <= i <= ntok + 3:
                emit_drainblock(i - 4)
            if 5 <= i <= ntok + 4:
                emit_ytail(i - 5)
            if i < ntok:
                emit_load(i)
            if 1 <= i <= ntok:
                emit_convert(i - 1)
            if 2 <= i <= ntok + 1:
                emit_tlo(i - 2)
            if 3 <= i <= ntok + 2:
                po = pout.tile([P, nfc, P], F32)
                emit_mm(i - 3, po)
            if i >= 8:
                del state[i - 8]

    nc.compile()
    return nc


_CACHE = {}


def _get_program():
    if "nc" not in _CACHE:
        _CACHE["nc"] = build()
    return _CACHE["nc"]


def kernel(x, weight_q, scale, bias):
    from concourse.bass_utils import run_bass_kernel_spmd

    try:
        import jax

        jax.config.update("jax_compilation_cache_dir", "/root/problem/jax_cache")
        jax.config.update("jax_persistent_cache_min_compile_time_secs", 0)
    except Exception:
        pass

    nc = _get_program()
    xr = np.ascontiguousarray(np.asarray(x, dtype=np.float32).reshape(TOK, IN))
    wq = np.asarray(weight_q, dtype=np.int32)
    sc = np.asarray(scale, dtype=np.float32)
    bi = np.asarray(bias, dtype=np.float32)
    in_maps = []
    for c in range(NCORES):
        f0 = c * FEAT
        in_maps.append({
            "x": xr,
            "wq": np.ascontiguousarray(wq[f0:f0 + FEAT].astype(np.uint8)),
            "scale": np.ascontiguousarray(sc[f0:f0 + FEAT]),
            "bias": np.ascontiguousarray(bi[f0:f0 + FEAT]),
        })
    res = run_bass_kernel_spmd(nc, in_maps, list(range(NCORES))).results
    y = np.concatenate([res[c]["y"][:FEAT].T for c in range(NCORES)], axis=1)
    return np.ascontiguousarray(y).reshape(B, S, OUT)


# revision 103
# speedup vs baseline: 1.6042x; 1.0267x over previous
"""Int4-quantized column-parallel linear (LLaMA-7B FFN up-proj) on 8 TRN2 cores.

y[b,s,o] = sum_i x[b,s,i] * (unpack_int4(weight_q)[o,i] * scale[o]) + bias[o]

Strategy (per core, 1/8 of out_features = 1376):
  - int4 weight nibbles are exact in fp8 e4m3; x is split x ~= hi + lo with
    hi = fp8(x), lo = fp8(x - hi), so the pair of fp8 matmuls reconstructs x
    to ~8e-4 relative. The lo-correction is skipped for LO_SKIP of the 16
    k-steps (residual error 2.65%*sqrt(LO_SKIP/16), far inside the 2e-2 gate),
    saving 1/32 of PE cycles per skipped step.
  - all matmuls run in MatmulPerfMode.DoubleRow (fp8 double-pumping): one
    instruction contracts 2x128 K rows at 0.5 PE cycles per output row.
  - layout: hi8/lo8 are converted straight from f32 x (no fp16 step), then
    DMA-transposed as uint16 byte-pairs; the resulting pair-interleaved
    [k-pair, 2, tok] operand rides the MOVING side of the matmul (the ISA
    allows strided moving APs but not strided LDWEIGHTS), with the weights
    pre-deinterleaved once into the matching pair-contiguous stationary
    layout. Output lands PSUM-[feat, tok], is drained with a fused per-
    partition scale*psum+bias tensor_scalar to f32 and DMA'd out still
    feat-major; the host-side unshard transposes to [tok, feat].
  - weight_q is shipped to the cores as uint8 (lossless re-encode of the
    packed byte values) to shrink the weight DMA and staging pools 4x.
  - all DMA transposes share the ACT hwdge queue (concurrent transposes on
    different queues corrupt each other on silicon), and every access made
    through a bitcast view carries an explicit dependency edge because the
    tile framework's tracker does not see them.
"""

from contextlib import ExitStack

import numpy as np

import concourse.bass as bass
import concourse.tile as tile
from concourse import bacc, mybir

F32 = mybir.dt.float32
F16 = mybir.dt.float16
F8 = mybir.dt.float8e4
I32 = mybir.dt.int32
U16 = mybir.dt.uint16
U8 = mybir.dt.uint8

B, S, IN, OUT = 4, 2048, 4096, 11008
NCORES = 8
TOK = B * S
FEAT = OUT // NCORES

P = 128
KP = IN // P          # 32 k-subtiles of 128
KSTEP = KP // 2       # 16 DoubleRow k-steps of 256
NTOK = TOK // P       # 64 token tiles
HALF = IN // 2

# Number of 256-row k-steps whose lo-correction matmul is skipped (of KSTEP).
LO_SKIP = 3

# Pool depths for the x pipeline (referenced by the manual WAR dep edges).
HI_BUFS = 2
LO_BUFS = 4
XT_BUFS = 6
XTL_BUFS = 4
WB_BUFS = 2
WSTG_BUFS = 1


def _dep(from_inst, to_inst, reason):
    tile.add_dep_helper(from_inst.ins, to_inst.ins, reason=reason)


def build(tok=TOK, in_dim=IN, feat=FEAT, debug_xt=False):
    assert tok % P == 0 and in_dim % 256 == 0
    ntok = tok // P
    kp = in_dim // P
    kstep = kp // 2
    half = in_dim // 2
    fchunks = [(c, c * P, min(P, feat - c * P)) for c in range((feat + P - 1) // P)]
    nfc = len(fchunks)

    nc = bacc.Bacc("TRN2", target_bir_lowering=False, debug=False,
                   num_devices=NCORES)
    x_d = nc.dram_tensor("x", [tok, in_dim], F32, kind="ExternalInput").ap()
    wq_d = nc.dram_tensor("wq", [feat, half], U8, kind="ExternalInput").ap()
    sc_d = nc.dram_tensor("scale", [feat], F32, kind="ExternalInput").ap()
    bi_d = nc.dram_tensor("bias", [feat], F32, kind="ExternalInput").ap()
    nfc_ = (feat + P - 1) // P
    y_d = nc.dram_tensor("y", [nfc_ * P, tok], F32, kind="ExternalOutput").ap()
    if debug_xt:
        dxh_d = nc.dram_tensor("dxh", [ntok, P, kp // 2, P], U16,
                               kind="ExternalOutput").ap()
        dxl_d = nc.dram_tensor("dxl", [ntok, P, kp // 2, P], U16,
                               kind="ExternalOutput").ap()

    with tile.TileContext(nc) as tc, ExitStack() as ctx:
        const = ctx.enter_context(tc.tile_pool(name="const", bufs=1))
        wtp = ctx.enter_context(tc.tile_pool(name="wt", bufs=1))
        wstg = ctx.enter_context(tc.tile_pool(name="wstg", bufs=WSTG_BUFS))
        wqp = ctx.enter_context(tc.tile_pool(name="wqp", bufs=5))
        wbp = ctx.enter_context(tc.tile_pool(name="wbp", bufs=WB_BUFS))
        wtmp = ctx.enter_context(tc.tile_pool(name="wtmp", bufs=2))
        x32p = ctx.enter_context(tc.tile_pool(name="x32", bufs=2))
        hip = ctx.enter_context(tc.tile_pool(name="hi8", bufs=HI_BUFS))
        lop = ctx.enter_context(tc.tile_pool(name="lo8", bufs=LO_BUFS))
        xthp = ctx.enter_context(tc.tile_pool(name="xth", bufs=XT_BUFS))
        xtlp = ctx.enter_context(tc.tile_pool(name="xtl", bufs=XTL_BUFS))
        otp = ctx.enter_context(tc.tile_pool(name="ot32", bufs=3))
        pout = ctx.enter_context(tc.tile_pool(name="pout", bufs=2, space="PSUM"))

        # scale/bias arranged (partition, chunk): [p, c] = value[128c + p];
        # pad partitions of the last chunk are zeroed so every chunk drains a
        # full 128 partitions from initialized memory
        scale_sb = const.tile([P, nfc], F32)
        bias_sb = const.tile([P, nfc], F32)
        lastsz = feat - (nfc - 1) * P
        for src, dst in ((sc_d, scale_sb), (bi_d, bias_sb)):
            if lastsz < P:
                nc.vector.memset(dst[lastsz:, nfc - 1:nfc], 0.0)
            nc.sync.dma_start(
                out=dst[:, :nfc - 1],
                in_=src[:(nfc - 1) * P].rearrange("(c p) -> p c", p=P))
            nc.sync.dma_start(
                out=dst[:lastsz, nfc - 1:nfc],
                in_=src[(nfc - 1) * P:].rearrange("(c p) -> p c", c=1))

        # Stationary weights, pair-contiguous: [k-pair(part), kblk, i, feat]
        # where element (p, j, i, f) = w[f, 256j + 2p + i]. The feat axis is
        # padded to nfc*128 with zero weights so every matmul/drain covers a
        # full 128 output partitions (cost is per output ROW, so this is free)
        feat_pad = nfc * P
        wT8x = wtp.tile([P, kstep, 2, feat_pad], F8)
        if feat_pad > feat:
            nc.gpsimd.memset(wT8x[:, :, :, feat:], 0.0)

        # ---- Phase W: unpack -> fp8 -> u16-pair transpose -> deinterleave ----
        # Nibbles sign-extended in 2 fused ALU ops ((q<<28)>>a28 / (q<<24)>>a28)
        # with the int32 ALU result converted to fp8 by the strided write.
        wq_tiles = {}

        def emit_wload(ftidx, f0, fsz):
            # all weight loads ride the otherwise-idle Pool queue so they are
            # never stuck behind a 6us x-tile load on SP
            wqt = wqp.tile([P, half], U8)
            nc.gpsimd.dma_start(out=wqt[:fsz], in_=wq_d[f0:f0 + fsz])
            wq_tiles[ftidx] = wqt

        wphase = {}

        def emit_wtile(ftidx, f0, fsz):
            wqt = wq_tiles.pop(ftidx)
            wb8 = wbp.tile([P, in_dim], F8)
            wb8v = wb8[:fsz].rearrange("p (i two) -> p two i", two=2)
            # bitVec TSP ops cannot cast and only run on DVE; bias the nibbles
            # in int32 there, then convert+subtract-8 on ACT (Copy, float bias)
            n_lo = wtmp.tile([P, half], U8)
            nc.vector.tensor_scalar(
                out=n_lo[:fsz], in0=wqt[:fsz], scalar1=15, scalar2=8,
                op0=mybir.AluOpType.bitwise_and, op1=mybir.AluOpType.bitwise_xor)
            n_hi = wtmp.tile([P, half], U8)
            nc.vector.tensor_scalar(
                out=n_hi[:fsz], in0=wqt[:fsz], scalar1=4, scalar2=8,
                op0=mybir.AluOpType.logical_shift_right,
                op1=mybir.AluOpType.bitwise_xor)
            c0 = nc.scalar.activation(out=wb8v[:, 0], in_=n_lo[:fsz],
                                      func=mybir.ActivationFunctionType.Copy,
                                      bias=-8.0)
            c1 = nc.scalar.activation(out=wb8v[:, 1], in_=n_hi[:fsz],
                                      func=mybir.ActivationFunctionType.Copy,
                                      bias=-8.0)
            if ftidx >= WB_BUFS:  # WAR vs the bitcast transpose read
                _dep(c0, wphase[ftidx - WB_BUFS]["wtr"],
                     reason="wb8 buffer WAR vs bitcast read")
                _dep(c1, wphase[ftidx - WB_BUFS]["wtr"],
                     reason="wb8 buffer WAR vs bitcast read")
            stg = wstg.tile([P, kstep, P], U16)
            # all DmaTransposeAnt share the ACT queue: concurrent transposes
            # on different hwdge queues corrupt each other (shared xbar)
            wtr = nc.scalar.dma_start_transpose(out=stg[:, :, :fsz],
                                                in_=wb8[:fsz].bitcast(U16))
            _dep(wtr, c0, reason="w transpose reads wb8 bitcast")
            _dep(wtr, c1, reason="w transpose reads wb8 bitcast")
            if ftidx >= WSTG_BUFS:  # WAR vs the bitcast deint reads
                for d in wphase[ftidx - WSTG_BUFS]["deints"]:
                    _dep(wtr, d, reason="stg buffer WAR vs bitcast read")
            # deinterleave the (k, k+1) byte pairs into the stationary layout
            sv = stg[:].rearrange("p a b -> p (a b)").bitcast(F8).rearrange(
                "p (j f two) -> p j two f", j=kstep, two=2)
            deints = []
            for b in range(2):
                d = nc.gpsimd.tensor_copy(out=wT8x[:, :, b, f0:f0 + fsz],
                                          in_=sv[:, :, b, :fsz])
                _dep(d, wtr, reason="deint reads stg bitcast")
                deints.append(d)
            wphase[ftidx] = {"wtr": wtr, "deints": deints}

        # All DmaTransposeAnt instructions are serialized through a global
        # dep chain so no two ever overlap (shared-xbar corruption); this
        # lets the back-transpose ride the otherwise idle SP queue.
        tchain = {"last": None}

        def _chain_transpose(t):
            if tchain["last"] is not None:
                _dep(t, tchain["last"], reason="xbar serialization chain")
            tchain["last"] = t

        # ---- Main loop stages ----
        # The dependency tracker does not see accesses made through bitcast
        # views, so every such read/write gets an explicit add_dep_helper edge
        # (RAW: view-reader after producer; WAR: buffer re-writer after the
        # last view-reader).
        state = {}

        def emit_load(i):
            x32 = x32p.tile([P, in_dim], F32)
            nc.sync.dma_start(out=x32[:], in_=x_d[i * P:(i + 1) * P])
            state[i] = {"x32": x32}

        def emit_convert(i):
            st = state[i]
            x32 = st["x32"]
            hi8 = hip.tile([P, in_dim], F8)
            cvt = nc.scalar.activation(out=hi8[:], in_=x32[:],
                                       func=mybir.ActivationFunctionType.Copy)
            on_dve = False
            if i >= HI_BUFS:
                _dep(cvt, state[i - HI_BUFS]["thi"],
                     reason="hi8 buffer WAR vs bitcast T_hi read")
            st["hi8"], st["cvt"] = hi8, cvt
            if not on_dve:
                # T_hi issued on ACT right after the ACT cvt (same engine, no
                # sequencer stall); RAW through the bitcast view
                emit_thi(i)
            lo8 = lop.tile([P, in_dim], F8)
            # only k >= LO_SKIP*256 of the residual is ever transposed/used
            sk = LO_SKIP * 256
            sub = nc.vector.tensor_tensor(out=lo8[:, sk:], in0=x32[:, sk:],
                                          in1=hi8[:, sk:],
                                          op=mybir.AluOpType.subtract)
            if i >= LO_BUFS:
                _dep(sub, state[i - LO_BUFS]["tlo"],
                     reason="lo8 buffer WAR vs bitcast T_lo read")
            st["lo8"], st["sub"] = lo8, sub

        def emit_thi(i):
            st = state[i]
            xth = xthp.tile([P, kstep, P], U16)
            thi = nc.scalar.dma_start_transpose(out=xth[:],
                                                in_=st["hi8"][:].bitcast(U16))
            _dep(thi, st["cvt"], reason="T_hi reads hi8 via bitcast")
            if i >= XT_BUFS:
                _dep(thi, state[i - XT_BUFS]["mm_last"],
                     reason="xth buffer WAR vs bitcast readers")
            if debug_xt:
                nc.gpsimd.dma_start(out=dxh_d[i], in_=xth[:])
            st["xth"] = xth
            st["thi"] = thi

        def emit_tlo(i):
            # lo transpose one iteration later: lo8(i) is long done, so the
            # ACT sequencer never blocks on the DVE semaphore
            st = state[i]
            if "xth" not in st:
                emit_thi(i)  # DVE-converted tiles get their T_hi here too
            xtl = xtlp.tile([P, kstep - LO_SKIP, P], U16)
            tlo = nc.scalar.dma_start_transpose(
                out=xtl[:], in_=st["lo8"][:, LO_SKIP * 256:].bitcast(U16))
            _dep(tlo, st["sub"], reason="T_lo reads lo8 via bitcast")
            if i >= XTL_BUFS:
                _dep(tlo, state[i - XTL_BUFS]["mm_last"],
                     reason="xtl buffer WAR vs bitcast readers")
            if debug_xt:
                nc.gpsimd.dma_start(out=dxl_d[i], in_=xtl[:])
            st["xtl"] = xtl
            st["tlo"] = tlo

        def emit_mm(i, po):
            st = state[i]
            st["po"] = po
            vh = st["xth"][:].rearrange("p a b -> p (a b)").bitcast(F8).rearrange(
                "p (j t two) -> p j two t", j=kstep, two=2)
            vl = st["xtl"][:].rearrange("p a b -> p (a b)").bitcast(F8).rearrange(
                "p (j t two) -> p j two t", j=kstep - LO_SKIP, two=2)
            first = None
            last = None
            for c, f0, fsz in fchunks:
                for j in range(kstep):
                    for b, v in ((0, vh), (1, vl)):
                        if b == 1 and j < LO_SKIP:
                            continue
                        last = nc.tensor.matmul(
                            out=po[:, c, :],
                            lhsT=wT8x[:, j, :, f0:f0 + P],
                            rhs=v[:, j - (LO_SKIP if b else 0), :, :],
                            start=(j == 0 and b == 0),
                            stop=(j == kstep - 1 and b == 1),
                            perf_mode=mybir.MatmulPerfMode.DoubleRow)
                        if first is None:
                            first = last
            # RAW: matmuls read xth/xtl via bitcast views (PE is in-order, so
            # an edge on the first matmul covers the whole tile)
            _dep(first, st["thi"], reason="mm reads xth bitcast")
            _dep(first, st["tlo"], reason="mm reads xtl bitcast")
            st["mm_last"] = last

        def emit_drainblock(i):
            # one iteration after the matmuls: all chunk stops are long past,
            # so these head-of-stream DVE ops run immediately and release the
            # PSUM buffer well before its next writer needs it
            st = state[i]
            po = st["po"]
            ot32 = otp.tile([P, nfc, P], F32)
            st["ot32"] = ot32
            for c, f0, fsz in fchunks:
                nc.vector.tensor_scalar(
                    out=ot32[:, c, :], in0=po[:, c, :],
                    scalar1=scale_sb[:, c:c + 1],
                    scalar2=bias_sb[:, c:c + 1],
                    op0=mybir.AluOpType.mult, op1=mybir.AluOpType.add)

        def emit_ytail(i):
            # output stays [feat, tok]-oriented; the host unshard transposes.
            # One strided DMA per tile: runs of 128 f32 (512B descriptors)
            ot32 = state[i]["ot32"]
            dst = y_d[:, i * P:(i + 1) * P].rearrange("(c p) t -> p c t", p=P)
            nc.gpsimd.dma_start(out=dst, in_=ot32[:])

        wdims = [(c * P, min(P, feat - c * P)) for c in range(nfc)]
        for ftidx in range(nfc + 5):
            if ftidx < nfc:
                emit_wload(ftidx, *wdims[ftidx])
            if ftidx >= 5:
                emit_wtile(ftidx - 5, *wdims[ftidx - 5])

        for i in range(ntok + 6):
            if 4 <= i <= ntok + 3:
                emit_drainblock(i - 4)
            if 5 <= i <= ntok + 4:
                emit_ytail(i - 5)
            if i < ntok:
                emit_load(i)
            if 1 <= i <= ntok:
                emit_convert(i - 1)
            if 2 <= i <= ntok + 1:
                emit_tlo(i - 2)
            if 3 <= i <= ntok + 2:
                po = pout.tile([P, nfc, P], F32)
                emit_mm(i - 3, po)
            if i >= 8:
                del state[i - 8]

    nc.compile()
    return nc


_CACHE = {}


def _get_program():
    if "nc" not in _CACHE:
        _CACHE["nc"] = build()
    return _CACHE["nc"]


def kernel(x, weight_q, scale, bias):
    from concourse.bass_utils import run_bass_kernel_spmd

    try:
        import jax

        jax.config.update("jax_compilation_cache_dir", "/root/problem/jax_cache")
        jax.config.update("jax_persistent_cache_min_compile_time_secs", 0)
    except Exception:
        pass

    nc = _get_program()
    xr = np.ascontiguousarray(np.asarray(x, dtype=np.float32).reshape(TOK, IN))
    wq = np.asarray(weight_q, dtype=np.int32)
    sc = np.asarray(scale, dtype=np.float32)
    bi = np.asarray(bias, dtype=np.float32)
    in_maps = []
    for c in range(NCORES):
        f0 = c * FEAT
        in_maps.append({
            "x": xr,
            "wq": np.ascontiguousarray(wq[f0:f0 + FEAT].astype(np.uint8)),
            "scale": np.ascontiguousarray(sc[f0:f0 + FEAT]),
            "bias": np.ascontiguousarray(bi[f0:f0 + FEAT]),
        })
    res = run_bass_kernel_spmd(nc, in_maps, list(range(NCORES))).results
    y = np.concatenate([res[c]["y"][:FEAT].T for c in range(NCORES)], axis=1)
    return np.ascontiguousarray(y).reshape(B, S, OUT)
